# revision 20
# baseline (speedup 1.0000x reference)
"""NonLocalBlock (dense self-attention over 64x64 pixels) on 8 Trainium2 cores.

Sharding: 8 cores = 4 batches x 2 query-halves of 2048 pixels each.
Each core holds the full x[b] (for keys/values) plus its query slice, and
computes its [C, 2048] slab of the output, residual included. The host
gathers the 8 slabs.

Scores stay float32r (TF32-like, full PE rate) — the attention here is
extremely peaked (scores span ~[-101,111]); bf16/fp8 scores measurably
break the 2e-2 tolerance. The PV side, the softmax denominator tree, and
g are bf16 (measured 1.9e-3 end-to-end): exp outputs bf16 directly, the
pair-sums run on DVE and quad-sums on GpSimd at 16-bit rates, and the
[P,P] ones matmul contracts each quad into s in fp32 PSUM.

Per-core math (layouts chosen so nothing is ever transposed):
  Q    = W_theta @ xq + b_theta          [256, 2048]  f32r
  phi  = W_phi   @ xf + b_phi            [256, 4096]  f32r
  g    = xf^T @ W_g^T                    [4096, 256]  bf16 (pixel on parts)
  fT[k,q] = sum_o phi[o,k] Q[o,q]        (PSUM, f32)
  eT   = exp(fT - 50)  -> bf16           (fixed shift: scores span ~[-101,111],
                                          bf16's e8 exponent holds e^61 fine)
  y~T[o,q] = sum_k g[k,o] eT[k,q]        (PV, bf16 operands, f32 PSUM accum)
  s[q]  = ones @ (quad-sums of eT)       (pairs on DVE, quads on GpSimd)
  out[c,q] = W_out^T (y~T * (1/s)) + b_out_eff[c] + x[c,q]
  where b_out_eff = b_out + W_out @ b_g

Schedule (the baseline lost ~80us to pipeline bubbles):
  * x arrives in eight 1MB column-slices ([128, 4, 512] each); the QKV
    phase is a t-slice loop (Q, phi, g per slice) so compute starts after
    ~2 slices instead of after the whole 8.4MB load.
  * ~14 dummy matmuls on a zeroed scratch tile warm the PE HAM clock gate
    (cold PE runs at 1.2 GHz for its first ~3.4us) during the initial DMA.
  * 1/s (DVE iterative divide, 3.4us) is split in half and scheduled at
    the head of the next q-tile so it never gates anything but the tail.
  * Each q-tile's epilogue (recip, y*recip casts, out-proj matmuls,
    bias+residual, store) is emitted interleaved into the NEXT q-tile's
    first ten key-chunk iterations, so the PE never drains at q-tile
    boundaries.
One PSUM pool: qkv/y_acc rotation 4 banks + scores 2 + s_acc 1 + wy 1 = 8.
"""

import json

import numpy as np

B, C, HH, WW = 4, 512, 64, 64
CI = 256
N = HH * WW          # 4096 pixels
NQ = N // 2          # queries per core
P = 128
QT = 512             # q-tile width
NQT = NQ // QT       # 4 q-tiles per core
NKC = N // P         # 32 key chunks
NPAIR = NKC // 2     # 16 key-chunk pairs
NQUAD = NKC // 4     # 8 key-chunk quads
NCC = C // P         # 4 channel chunks
NOC = CI // P        # 2 inter-channel chunks
NT = N // QT         # 8 column slices of x
SHIFT = 50.0
NWARM = 20           # PE warm-up matmuls during the initial DMA

_cache: dict = {}


def _install_bir_patch():
    """This walrus build rejects >1 sync-wait per instruction; Tile's tail
    drain (and some first-consumer instructions) carry several. Split the
    extras onto preceding single-wait EventSemaphore instructions."""
    import concourse.bass_utils as bass_utils
    import concourse.bass2jax as bass2jax

    if getattr(bass_utils.compile_bir_kernel, "_wait_split_patch", False):
        return
    orig = bass_utils.compile_bir_kernel

    def _split(bir_json: bytes) -> bytes:
        d = json.loads(bir_json)
        changed = False
        for fn in d.get("functions", []):
            for bb in fn.get("blocks", []):
                new = []
                for ins in bb.get("instructions", []):
                    si = ins.get("sync_info")
                    waits = (si or {}).get("on_wait") or []
                    if len(waits) > 1:
                        changed = True
                        for k, w in enumerate(waits[:-1]):
                            new.append({
                                "debug": ins.get("debug", 0),
                                "engine": ins["engine"],
                                "ins": [],
                                "outs": [],
                                "name": f"{ins['name']}-w{k}",
                                "opcode": "EventSemaphore",
                                "sync_info": {"on_update": [], "on_wait": [w]},
                            })
                        si["on_wait"] = [waits[-1]]
                    new.append(ins)
                bb["instructions"] = new
        return json.dumps(d).encode() if changed else bir_json

    def patched(bir_json, tmpdir, neff_name="file.neff"):
        return orig(_split(bir_json), tmpdir, neff_name)

    patched._wait_split_patch = True
    bass_utils.compile_bir_kernel = patched
    bass2jax.compile_bir_kernel = patched


def _build_nc():
    import concourse.bass as bass
    import concourse.mybir as mybir
    from concourse import tile
    from concourse.alu_op_type import AluOpType

    dt = mybir.dt
    f32, f32r, bf16 = dt.float32, dt.float32r, dt.bfloat16
    Exp = mybir.ActivationFunctionType.Exp

    nc = bass.Bass("TRN2", target_bir_lowering=False, debug=False)

    xf_d = nc.dram_tensor("xf", [C, N], f32, kind="ExternalInput")
    wqp_d = nc.dram_tensor("wqp", [C, 2 * CI], f32, kind="ExternalInput")
    wg_d = nc.dram_tensor("wg", [C, CI], bf16, kind="ExternalInput")
    wo_d = nc.dram_tensor("wo", [CI, C], f32, kind="ExternalInput")
    bqp_d = nc.dram_tensor("bqp", [P, 6], f32, kind="ExternalInput")
    bo_d = nc.dram_tensor("bo", [P, NCC], f32, kind="ExternalInput")
    ones_d = nc.dram_tensor("ones", [P, P], bf16, kind="ExternalInput")
    out_d = nc.dram_tensor("out", [C, NQ], f32, kind="ExternalOutput")

    with tile.TileContext(nc) as tc:
        with (
            tc.tile_pool(name="wts", bufs=1) as wpool,
            tc.tile_pool(name="persist", bufs=1) as ppool,
        ):
            wqp_s = wpool.tile([P, NCC, 2 * CI], f32r)
            wg_s = wpool.tile([P, NCC, CI], bf16)
            wo_s = wpool.tile([P, NOC, C], f32r)
            bqp_s = wpool.tile([P, 6], f32)
            bo_s = wpool.tile([P, NCC], f32)
            ones_s = wpool.tile([P, P], bf16)
            scr_s = wpool.tile([P, 2 * P], f32)

            q_s = ppool.tile([P, NOC, NQ], f32r)
            phi_s = ppool.tile([P, NOC, N], f32r)
            g_s = ppool.tile([P, NKC, CI], bf16)
            xf_s = ppool.tile([P, NCC, N], f32r)
            xb_s = ppool.tile([P, NCC, N], bf16)

            all_ps = tc.tile_pool(name="all_ps", bufs=1, space="PSUM")
            ctx_ps = all_ps.__enter__()

            # ---- warm-up: keep the PE busy during the x DMA so the HAM
            # clock gate releases before real work starts ----
            nc.vector.memset(scr_s[:], 0.0)
            for i in range(NWARM):
                wp = ctx_ps.tile([P, QT], f32, tag="fps", bufs=2, name="warm")
                nc.tensor.matmul(wp[:, :2 * P], scr_s[:, :P].bitcast(f32r),
                                 scr_s[:].bitcast(f32r), start=True, stop=True)

            # ---- input DMAs. dma_start ISSUE costs ~0.7-2.4us each on its
            # engine, so x slices issue from Sync while all weights issue in
            # parallel from the (otherwise idle at startup) Scalar engine ----
            xf_r = xf_d.ap().rearrange("(kc p) n -> p kc n", p=P).bitcast(f32r)
            wqp_r = wqp_d.ap().rearrange("(kc p) m -> p kc m", p=P).bitcast(f32r)
            xf_raw = xf_d.ap().rearrange("(kc p) n -> p kc n", p=P)
            nc.scalar.dma_start(bqp_s[:], bqp_d.ap())
            nc.scalar.dma_start(wqp_s[:, :, 0:CI], wqp_r[:, :, 0:CI])
            for t in range(NT):
                nc.sync.dma_start(xf_s[:, :, t * QT:(t + 1) * QT],
                                  xf_r[:, :, t * QT:(t + 1) * QT])
            nc.scalar.dma_start(wqp_s[:, :, CI:], wqp_r[:, :, CI:])
            nc.scalar.dma_start(wg_s[:], wg_d.ap().rearrange("(kc p) o -> p kc o", p=P))
            for t in range(NT):
                # f32 -> bf16 casting DMA (gpsimd-only feature): a bf16 copy
                # of x whose slices serve as the g-matmul stationaries (bf16
                # LDWEIGHTS gets fast-weight-load; f32r LDW at 190ns cannot
                # hide under g's 107ns matmuls)
                nc.gpsimd.dma_start(xb_s[:, :, t * QT:(t + 1) * QT],
                                    xf_raw[:, :, t * QT:(t + 1) * QT])
            nc.scalar.dma_start(ones_s[:], ones_d.ap())
            nc.scalar.dma_start(wo_s[:], wo_d.ap().rearrange("(oc p) c -> p oc c", p=P).bitcast(f32r))
            nc.scalar.dma_start(bo_s[:], bo_d.ap())

            # ---- QKV phase: one pass per 512-column slice of x ----
            for t in range(NT):
                tsl = slice(t * QT, (t + 1) * QT)
                if t < NQ // QT:
                    for mc in range(NOC):
                        ps = ctx_ps.tile([P, QT], f32, tag="qkv", bufs=4, name="psq")
                        for kc in range(NCC):
                            nc.tensor.matmul(
                                ps[:],
                                wqp_s[:, kc, mc * P:(mc + 1) * P],
                                xf_s[:, kc, tsl],
                                start=(kc == 0),
                                stop=(kc == NCC - 1),
                            )
                        nc.vector.tensor_scalar_add(
                            q_s[:, mc, tsl], ps[:], bqp_s[:, mc:mc + 1])
                for mc in range(NOC):
                    ps = ctx_ps.tile([P, QT], f32, tag="qkv", bufs=4, name="psp")
                    for kc in range(NCC):
                        nc.tensor.matmul(
                            ps[:],
                            wqp_s[:, kc, (NOC + mc) * P:(NOC + mc + 1) * P],
                            xf_s[:, kc, tsl],
                            start=(kc == 0),
                            stop=(kc == NCC - 1),
                        )
                    nc.vector.tensor_scalar_add(
                        phi_s[:, mc, tsl], ps[:], bqp_s[:, NOC + mc:NOC + mc + 1])
                for kc in range(4 * t, 4 * t + 4):
                    ps = ctx_ps.tile([P, QT], f32, tag="qkv", bufs=4, name="psg")[:, :CI]
                    for cc in range(NCC):
                        nc.tensor.matmul(
                            ps[:],
                            xb_s[:, cc, kc * P:(kc + 1) * P],
                            wg_s[:, cc, :],
                            start=(cc == 0),
                            stop=(cc == NCC - 1),
                        )
                    nc.vector.tensor_copy(g_s[:, kc, :], ps[:])

            # ---- attention + interleaved epilogues ----
            with (
                tc.tile_pool(name="attn_sb", bufs=3) as apool,
                tc.tile_pool(name="epi_sb", bufs=2) as epool,
            ):
                def emit_epilogue(prev, step):
                    """One piece of the previous q-tile's epilogue, scattered
                    over the next q-tile's early key-chunk iterations (or run
                    back-to-back for the final q-tile). 1/s = exp(-ln(s)) on
                    the Scalar engine: two 0.7us ACT ops instead of the 3.4us
                    DVE iterative divide (ACT table error ~1e-4 rel; s spans
                    ~[8e-4, 2e30], well inside both tables)."""
                    if step == 0:
                        # ACT Ln returns garbage above 2^64, so fold a 2^-40
                        # prescale into its input; the Exp bias compensates
                        lns = epool.tile([P, QT], f32, tag="lns", bufs=1,
                                         name="lns")
                        nc.scalar.activation(
                            lns[:], prev["s_acc"][:],
                            mybir.ActivationFunctionType.Ln, scale=2.0 ** -40)
                        prev["lns"] = lns
                    elif step == 1:
                        recip = epool.tile([P, QT], f32, tag="recip", bufs=1,
                                           name="recip")
                        nc.scalar.activation(recip[:], prev["lns"][:], Exp,
                                             scale=-1.0, bias=bqp_s[:, 5:6])
                        prev["recip"] = recip
                    elif step in (2, 3):
                        oc = step - 2
                        yT = epool.tile([P, QT], f32r, tag="yT", bufs=2, name="yT")
                        nc.vector.tensor_mul(
                            out=yT[:], in0=prev["y_acc"][oc][:], in1=prev["recip"][:])
                        prev["yT"][oc] = yT
                    else:
                        cc = step - 4
                        wy = ctx_ps.tile([P, QT], f32, tag="wy", bufs=1, name="wy")
                        for oc in range(NOC):
                            nc.tensor.matmul(
                                wy[:],
                                wo_s[:, oc, cc * P:(cc + 1) * P],
                                prev["yT"][oc][:],
                                start=(oc == 0),
                                stop=(oc == NOC - 1),
                            )
                        ot = epool.tile([P, QT], f32, tag="ot", bufs=2, name="ot")
                        nc.vector.scalar_tensor_tensor(
                            out=ot[:], in0=wy[:], scalar=bo_s[:, cc:cc + 1],
                            in1=xf_s[:, cc, prev["qsl"]].bitcast(f32),
                            op0=AluOpType.add, op1=AluOpType.add)
                        nc.sync.dma_start(
                            out_d.ap()[cc * P:(cc + 1) * P, prev["qsl"]], ot[:])

                prev = None
                for qt in range(NQT):
                    qsl = slice(qt * QT, (qt + 1) * QT)
                    last = qt == NQT - 1
                    y_acc = [ctx_ps.tile([P, QT], f32, tag="qkv", bufs=4,
                                         name=f"yacc{oc}") for oc in range(NOC)]
                    s_acc = ctx_ps.tile([P, QT], f32, tag="sacc", bufs=1,
                                        name="s_acc")
                    exps = [None] * NKC
                    esums = [None] * NPAIR
                    equads = [None] * NQUAD

                    def scores_exp(kc):
                        fp = ctx_ps.tile([P, QT], f32, tag="fps", bufs=2, name="fp")
                        for oc in range(NOC):
                            nc.tensor.matmul(
                                fp[:],
                                phi_s[:, oc, kc * P:(kc + 1) * P],
                                q_s[:, oc, qsl],
                                start=(oc == 0),
                                stop=(oc == NOC - 1),
                            )
                        eT = apool.tile([P, QT], bf16, tag="eT", bufs=5)
                        nc.scalar.activation(eT[:], fp[:], Exp, bias=bqp_s[:, 4:5])
                        exps[kc] = eT
                        if kc % 2 == 1:
                            eS = apool.tile([P, QT], bf16, tag="eS", bufs=4)
                            nc.vector.tensor_add(
                                out=eS[:], in0=exps[kc - 1][:], in1=eT[:])
                            esums[kc // 2] = eS
                            if kc % 4 == 3 and not (last and kc == NKC - 1):
                                eQ = apool.tile([P, QT], bf16, tag="eQ", bufs=3)
                                nc.gpsimd.tensor_add(
                                    out=eQ[:], in0=esums[kc // 2 - 1][:], in1=eS[:])
                                equads[kc // 4] = eQ
                                esums[kc // 2 - 1] = None
                                esums[kc // 2] = None

                    def pv_only(kc):
                        eT = exps[kc]
                        for oc in range(NOC):
                            nc.tensor.matmul(
                                y_acc[oc][:],
                                g_s[:, kc, oc * P:(oc + 1) * P],
                                eT[:],
                                start=(kc == 0),
                                stop=(kc == NKC - 1),
                                skip_group_check=True,
                            )
                        exps[kc] = None

                    def sums_quad(qq):
                        if last and qq == NQUAD - 1:
                            # final q-tile: sum the last two pairs directly so
                            # the tail's critical chain skips the 1.25us
                            # GpSimd quad-add (exp31 -> pair15 -> matmul)
                            for j, pp in enumerate((NPAIR - 2, NPAIR - 1)):
                                nc.tensor.matmul(
                                    s_acc[:],
                                    ones_s[:],
                                    esums[pp][:],
                                    start=False,
                                    stop=(j == 1),
                                    skip_group_check=True,
                                )
                                esums[pp] = None
                            return
                        nc.tensor.matmul(
                            s_acc[:],
                            ones_s[:],
                            equads[qq][:],
                            start=(qq == 0),
                            stop=(qq == NQUAD - 1),
                            skip_group_check=True,
                        )
                        equads[qq] = None

                    epi_sched = {1: 0, 3: 1, 5: 2, 6: 3, 7: 4, 9: 5, 11: 6, 13: 7}
                    for kc in range(NKC + 6):
                        if prev is not None and kc in epi_sched:
                            emit_epilogue(prev, epi_sched[kc])
                        if kc < NKC:
                            scores_exp(kc)
                        if 2 <= kc < NKC + 2:
                            pv_only(kc - 2)
                        if kc >= 9 and (kc - 9) % 4 == 0 and (kc - 9) // 4 < NQUAD:
                            sums_quad((kc - 9) // 4)

                    prev = {"y_acc": y_acc, "s_acc": s_acc, "qsl": qsl,
                            "yT": [None] * NOC}

                # final q-tile's epilogue has no successor to hide in
                for step in range(8):
                    emit_epilogue(prev, step)
            all_ps.__exit__(None, None, None)
    return nc


def _get_nc():
    if "nc" not in _cache:
        _install_bir_patch()
        _cache["nc"] = _build_nc()
    return _cache["nc"]


def kernel(x, w_theta, b_theta, w_phi, b_phi, w_g, b_g, w_out, b_out,
           _trace=False):
    import ml_dtypes
    from concourse.bass_utils import run_bass_kernel_spmd

    x = np.asarray(x, dtype=np.float32)
    w_theta = np.asarray(w_theta, dtype=np.float32)
    b_theta = np.asarray(b_theta, dtype=np.float32)
    w_phi = np.asarray(w_phi, dtype=np.float32)
    b_phi = np.asarray(b_phi, dtype=np.float32)
    w_g = np.asarray(w_g, dtype=np.float32)
    b_g = np.asarray(b_g, dtype=np.float32)
    w_out = np.asarray(w_out, dtype=np.float32)
    b_out = np.asarray(b_out, dtype=np.float32)

    nc = _get_nc()

    xf = np.ascontiguousarray(x.reshape(B, C, N))
    wqp = np.ascontiguousarray(np.concatenate([w_theta, w_phi], axis=0).T)  # [C, 2CI]
    wg = np.ascontiguousarray(w_g.T.astype(ml_dtypes.bfloat16))  # [C, CI] bf16
    wo = np.ascontiguousarray(w_out.T)                     # [CI, C]
    bqp = np.ascontiguousarray(
        np.stack([b_theta[:P], b_theta[P:], b_phi[:P], b_phi[P:],
                  np.full(P, -SHIFT, np.float32),
                  np.full(P, -40.0 * np.log(2.0), np.float32)], axis=1))  # [P, 6]
    bo_eff = b_out + w_out @ b_g
    bo = np.ascontiguousarray(bo_eff.reshape(NCC, P).T)    # [P, NCC]
    ones = np.ones((P, P), dtype=ml_dtypes.bfloat16)

    shared = {"wqp": wqp, "wg": wg, "wo": wo, "bqp": bqp, "bo": bo, "ones": ones}
    in_maps = []
    for core in range(8):
        b, h = divmod(core, 2)
        # query half first; attention is permutation-invariant over keys
        xperm = np.concatenate(
            [xf[b][:, h * NQ:(h + 1) * NQ], xf[b][:, (1 - h) * NQ:(2 - h) * NQ]],
            axis=1)
        in_maps.append({"xf": np.ascontiguousarray(xperm), **shared})

    res = run_bass_kernel_spmd(nc, in_maps, core_ids=list(range(8)), trace=_trace)
    _cache["last_results"] = res

    out = np.empty((B, C, N), dtype=np.float32)
    for core in range(8):
        b, h = divmod(core, 2)
        out[b][:, h * NQ:(h + 1) * NQ] = res.results[core]["out"]
    return out.reshape(B, C, HH, WW)


# revision 21
# speedup vs baseline: 1.0234x; 1.0234x over previous
"""NonLocalBlock (dense self-attention over 64x64 pixels) on 8 Trainium2 cores.

Sharding: 8 cores = 4 batches x 2 query-halves of 2048 pixels each.
Each core holds the full x[b] (for keys/values) plus its query slice, and
computes its [C, 2048] slab of the output, residual included. The host
gathers the 8 slabs.

Scores stay float32r (TF32-like, full PE rate) — the attention here is
extremely peaked (scores span ~[-101,111]); bf16/fp8 scores measurably
break the 2e-2 tolerance. The PV side, the softmax denominator tree, and
g are bf16 (measured 1.9e-3 end-to-end): exp outputs bf16 directly, the
pair-sums run on DVE and quad-sums on GpSimd at 16-bit rates, and the
[P,P] ones matmul contracts each quad into s in fp32 PSUM.

Per-core math (layouts chosen so nothing is ever transposed):
  Q    = W_theta @ xq + b_theta          [256, 2048]  f32r
  phi  = W_phi   @ xf + b_phi            [256, 4096]  f32r
  g    = xf^T @ W_g^T                    [4096, 256]  bf16 (pixel on parts)
  fT[k,q] = sum_o phi[o,k] Q[o,q]        (PSUM, f32)
  eT   = exp(fT - 50)  -> bf16           (fixed shift: scores span ~[-101,111],
                                          bf16's e8 exponent holds e^61 fine)
  y~T[o,q] = sum_k g[k,o] eT[k,q]        (PV, bf16 operands, f32 PSUM accum)
  s[q]  = ones @ (quad-sums of eT)       (pairs on DVE, quads on GpSimd)
  out[c,q] = W_out^T (y~T * (1/s)) + b_out_eff[c] + x[c,q]
  where b_out_eff = b_out + W_out @ b_g

Schedule (the baseline lost ~80us to pipeline bubbles):
  * x arrives in eight 1MB column-slices ([128, 4, 512] each); the QKV
    phase is a t-slice loop (Q, phi, g per slice) so compute starts after
    ~2 slices instead of after the whole 8.4MB load.
  * ~14 dummy matmuls on a zeroed scratch tile warm the PE HAM clock gate
    (cold PE runs at 1.2 GHz for its first ~3.4us) during the initial DMA.
  * 1/s (DVE iterative divide, 3.4us) is split in half and scheduled at
    the head of the next q-tile so it never gates anything but the tail.
  * Each q-tile's epilogue (recip, y*recip casts, out-proj matmuls,
    bias+residual, store) is emitted interleaved into the NEXT q-tile's
    first ten key-chunk iterations, so the PE never drains at q-tile
    boundaries.
One PSUM pool: qkv/y_acc rotation 4 banks + scores 2 + s_acc 1 + wy 1 = 8.
"""

import json

import numpy as np

B, C, HH, WW = 4, 512, 64, 64
CI = 256
N = HH * WW          # 4096 pixels
NQ = N // 2          # queries per core
P = 128
QT = 512             # q-tile width
NQT = NQ // QT       # 4 q-tiles per core
NKC = N // P         # 32 key chunks
NPAIR = NKC // 2     # 16 key-chunk pairs
NQUAD = NKC // 4     # 8 key-chunk quads
NCC = C // P         # 4 channel chunks
NOC = CI // P        # 2 inter-channel chunks
NT = N // QT         # 8 column slices of x
SHIFT = 50.0
NWARM = 22           # PE warm-up matmuls during the initial DMA

_cache: dict = {}


def _install_bir_patch():
    """This walrus build rejects >1 sync-wait per instruction; Tile's tail
    drain (and some first-consumer instructions) carry several. Split the
    extras onto preceding single-wait EventSemaphore instructions."""
    import concourse.bass_utils as bass_utils
    import concourse.bass2jax as bass2jax

    if getattr(bass_utils.compile_bir_kernel, "_wait_split_patch", False):
        return
    orig = bass_utils.compile_bir_kernel

    def _split(bir_json: bytes) -> bytes:
        d = json.loads(bir_json)
        changed = False
        for fn in d.get("functions", []):
            for bb in fn.get("blocks", []):
                new = []
                for ins in bb.get("instructions", []):
                    si = ins.get("sync_info")
                    waits = (si or {}).get("on_wait") or []
                    if len(waits) > 1:
                        changed = True
                        for k, w in enumerate(waits[:-1]):
                            new.append({
                                "debug": ins.get("debug", 0),
                                "engine": ins["engine"],
                                "ins": [],
                                "outs": [],
                                "name": f"{ins['name']}-w{k}",
                                "opcode": "EventSemaphore",
                                "sync_info": {"on_update": [], "on_wait": [w]},
                            })
                        si["on_wait"] = [waits[-1]]
                    new.append(ins)
                bb["instructions"] = new
        return json.dumps(d).encode() if changed else bir_json

    def patched(bir_json, tmpdir, neff_name="file.neff"):
        return orig(_split(bir_json), tmpdir, neff_name)

    patched._wait_split_patch = True
    bass_utils.compile_bir_kernel = patched
    bass2jax.compile_bir_kernel = patched


def _build_nc():
    import concourse.bass as bass
    import concourse.mybir as mybir
    from concourse import tile
    from concourse.alu_op_type import AluOpType

    dt = mybir.dt
    f32, f32r, bf16 = dt.float32, dt.float32r, dt.bfloat16
    Exp = mybir.ActivationFunctionType.Exp

    nc = bass.Bass("TRN2", target_bir_lowering=False, debug=False)

    xf_d = nc.dram_tensor("xf", [C, N], f32, kind="ExternalInput")
    wqp_d = nc.dram_tensor("wqp", [C, 2 * CI], f32, kind="ExternalInput")
    wg_d = nc.dram_tensor("wg", [C, CI], bf16, kind="ExternalInput")
    wo_d = nc.dram_tensor("wo", [CI, C], f32, kind="ExternalInput")
    bqp_d = nc.dram_tensor("bqp", [P, 6], f32, kind="ExternalInput")
    bo_d = nc.dram_tensor("bo", [P, NCC], f32, kind="ExternalInput")
    ones_d = nc.dram_tensor("ones", [P, P], bf16, kind="ExternalInput")
    out_d = nc.dram_tensor("out", [C, NQ], f32, kind="ExternalOutput")

    with tile.TileContext(nc) as tc:
        with (
            tc.tile_pool(name="wts", bufs=1) as wpool,
            tc.tile_pool(name="persist", bufs=1) as ppool,
        ):
            wqp_s = wpool.tile([P, NCC, 2 * CI], f32r)
            wg_s = wpool.tile([P, NCC, CI], bf16)
            wo_s = wpool.tile([P, NOC, C], f32r)
            bqp_s = wpool.tile([P, 6], f32)
            bo_s = wpool.tile([P, NCC], f32)
            ones_s = wpool.tile([P, P], bf16)
            scr_s = wpool.tile([P, 2 * P], f32)

            q_s = ppool.tile([P, NOC, NQ], f32r)
            phi_s = ppool.tile([P, NOC, N], f32r)
            g_s = ppool.tile([P, NKC, CI], bf16)
            xf_s = ppool.tile([P, NCC, N], f32r)
            xb_s = ppool.tile([P, NCC, N], bf16)

            all_ps = tc.tile_pool(name="all_ps", bufs=1, space="PSUM")
            ctx_ps = all_ps.__enter__()

            # ---- warm-up: keep the PE busy during the x DMA so the HAM
            # clock gate releases before real work starts ----
            nc.vector.memset(scr_s[:], 0.0)
            for i in range(NWARM):
                wp = ctx_ps.tile([P, QT], f32, tag="fps", bufs=2, name="warm")
                nc.tensor.matmul(wp[:, :2 * P], scr_s[:, :P].bitcast(f32r),
                                 scr_s[:].bitcast(f32r), start=True, stop=True)

            # ---- input DMAs. dma_start ISSUE costs ~0.7-2.4us each on its
            # engine, so x slices issue from Sync while all weights issue in
            # parallel from the (otherwise idle at startup) Scalar engine ----
            xf_r = xf_d.ap().rearrange("(kc p) n -> p kc n", p=P).bitcast(f32r)
            wqp_r = wqp_d.ap().rearrange("(kc p) m -> p kc m", p=P).bitcast(f32r)
            nc.scalar.dma_start(bqp_s[:], bqp_d.ap())
            nc.scalar.dma_start(wqp_s[:, :, 0:CI], wqp_r[:, :, 0:CI])
            for t in range(NT):
                nc.sync.dma_start(xf_s[:, :, t * QT:(t + 1) * QT],
                                  xf_r[:, :, t * QT:(t + 1) * QT])
            nc.scalar.dma_start(wqp_s[:, :, CI:], wqp_r[:, :, CI:])
            nc.scalar.dma_start(wg_s[:], wg_d.ap().rearrange("(kc p) o -> p kc o", p=P))
            for t in range(NT):
                # f32 -> bf16 casting DMA (gpsimd-only feature), SBUF-to-SBUF
                # so it adds no HBM traffic: the bf16 x copy serves as the
                # g-matmul stationaries (bf16 LDWEIGHTS gets fast-weight-load;
                # f32r LDW at 190ns cannot hide under g's 107ns matmuls)
                nc.gpsimd.dma_start(xb_s[:, :, t * QT:(t + 1) * QT],
                                    xf_s[:, :, t * QT:(t + 1) * QT].bitcast(f32))
            nc.scalar.dma_start(ones_s[:], ones_d.ap())
            nc.scalar.dma_start(wo_s[:], wo_d.ap().rearrange("(oc p) c -> p oc c", p=P).bitcast(f32r))
            nc.scalar.dma_start(bo_s[:], bo_d.ap())

            # ---- QKV phase: one pass per 512-column slice of x ----
            for t in range(NT):
                tsl = slice(t * QT, (t + 1) * QT)
                if t < NQ // QT:
                    for mc in range(NOC):
                        ps = ctx_ps.tile([P, QT], f32, tag="qkv", bufs=4, name="psq")
                        for kc in range(NCC):
                            nc.tensor.matmul(
                                ps[:],
                                wqp_s[:, kc, mc * P:(mc + 1) * P],
                                xf_s[:, kc, tsl],
                                start=(kc == 0),
                                stop=(kc == NCC - 1),
                            )
                        nc.vector.tensor_scalar_add(
                            q_s[:, mc, tsl], ps[:], bqp_s[:, mc:mc + 1])
                for mc in range(NOC):
                    ps = ctx_ps.tile([P, QT], f32, tag="qkv", bufs=4, name="psp")
                    for kc in range(NCC):
                        nc.tensor.matmul(
                            ps[:],
                            wqp_s[:, kc, (NOC + mc) * P:(NOC + mc + 1) * P],
                            xf_s[:, kc, tsl],
                            start=(kc == 0),
                            stop=(kc == NCC - 1),
                        )
                    nc.vector.tensor_scalar_add(
                        phi_s[:, mc, tsl], ps[:], bqp_s[:, NOC + mc:NOC + mc + 1])
                for kc in range(4 * t, 4 * t + 4):
                    ps = ctx_ps.tile([P, QT], f32, tag="qkv", bufs=4, name="psg")[:, :CI]
                    for cc in range(NCC):
                        nc.tensor.matmul(
                            ps[:],
                            xb_s[:, cc, kc * P:(kc + 1) * P],
                            wg_s[:, cc, :],
                            start=(cc == 0),
                            stop=(cc == NCC - 1),
                        )
                    nc.vector.tensor_copy(g_s[:, kc, :], ps[:])

            # ---- attention + interleaved epilogues ----
            with (
                tc.tile_pool(name="attn_sb", bufs=3) as apool,
                tc.tile_pool(name="epi_sb", bufs=2) as epool,
            ):
                def emit_epilogue(prev, step):
                    """One piece of the previous q-tile's epilogue, scattered
                    over the next q-tile's early key-chunk iterations (or run
                    back-to-back for the final q-tile). 1/s = exp(-ln(s)) on
                    the Scalar engine: two 0.7us ACT ops instead of the 3.4us
                    DVE iterative divide (ACT table error ~1e-4 rel; s spans
                    ~[8e-4, 2e30], well inside both tables)."""
                    if step == 0:
                        # ACT Ln returns garbage above 2^64, so fold a 2^-40
                        # prescale into its input; the Exp bias compensates
                        lns = epool.tile([P, QT], f32, tag="lns", bufs=1,
                                         name="lns")
                        nc.scalar.activation(
                            lns[:], prev["s_acc"][:],
                            mybir.ActivationFunctionType.Ln, scale=2.0 ** -40)
                        prev["lns"] = lns
                    elif step == 1:
                        recip = epool.tile([P, QT], f32, tag="recip", bufs=1,
                                           name="recip")
                        nc.scalar.activation(recip[:], prev["lns"][:], Exp,
                                             scale=-1.0, bias=bqp_s[:, 5:6])
                        prev["recip"] = recip
                    elif step in (2, 3):
                        oc = step - 2
                        yT = epool.tile([P, QT], f32r, tag="yT", bufs=2, name="yT")
                        nc.vector.tensor_mul(
                            out=yT[:], in0=prev["y_acc"][oc][:], in1=prev["recip"][:])
                        prev["yT"][oc] = yT
                    else:
                        cc = step - 4
                        wy = ctx_ps.tile([P, QT], f32, tag="wy", bufs=1, name="wy")
                        for oc in range(NOC):
                            nc.tensor.matmul(
                                wy[:],
                                wo_s[:, oc, cc * P:(cc + 1) * P],
                                prev["yT"][oc][:],
                                start=(oc == 0),
                                stop=(oc == NOC - 1),
                            )
                        ot = epool.tile([P, QT], f32, tag="ot", bufs=2, name="ot")
                        nc.vector.scalar_tensor_tensor(
                            out=ot[:], in0=wy[:], scalar=bo_s[:, cc:cc + 1],
                            in1=xf_s[:, cc, prev["qsl"]].bitcast(f32),
                            op0=AluOpType.add, op1=AluOpType.add)
                        nc.sync.dma_start(
                            out_d.ap()[cc * P:(cc + 1) * P, prev["qsl"]], ot[:])

                prev = None
                for qt in range(NQT):
                    qsl = slice(qt * QT, (qt + 1) * QT)
                    last = qt == NQT - 1
                    y_acc = [ctx_ps.tile([P, QT], f32, tag="qkv", bufs=4,
                                         name=f"yacc{oc}") for oc in range(NOC)]
                    s_acc = ctx_ps.tile([P, QT], f32, tag="sacc", bufs=1,
                                        name="s_acc")
                    exps = [None] * NKC
                    esums = [None] * NPAIR
                    equads = [None] * NQUAD

                    def scores_exp(kc):
                        fp = ctx_ps.tile([P, QT], f32, tag="fps", bufs=2, name="fp")
                        for oc in range(NOC):
                            nc.tensor.matmul(
                                fp[:],
                                phi_s[:, oc, kc * P:(kc + 1) * P],
                                q_s[:, oc, qsl],
                                start=(oc == 0),
                                stop=(oc == NOC - 1),
                            )
                        eT = apool.tile([P, QT], bf16, tag="eT", bufs=5)
                        nc.scalar.activation(eT[:], fp[:], Exp, bias=bqp_s[:, 4:5])
                        exps[kc] = eT
                        if kc % 2 == 1:
                            eS = apool.tile([P, QT], bf16, tag="eS", bufs=4)
                            nc.vector.tensor_add(
                                out=eS[:], in0=exps[kc - 1][:], in1=eT[:])
                            esums[kc // 2] = eS
                            if kc % 4 == 3 and not (last and kc == NKC - 1):
                                eQ = apool.tile([P, QT], bf16, tag="eQ", bufs=3)
                                nc.gpsimd.tensor_add(
                                    out=eQ[:], in0=esums[kc // 2 - 1][:], in1=eS[:])
                                equads[kc // 4] = eQ
                                esums[kc // 2 - 1] = None
                                esums[kc // 2] = None

                    def pv_only(kc):
                        eT = exps[kc]
                        for oc in range(NOC):
                            nc.tensor.matmul(
                                y_acc[oc][:],
                                g_s[:, kc, oc * P:(oc + 1) * P],
                                eT[:],
                                start=(kc == 0),
                                stop=(kc == NKC - 1),
                                skip_group_check=True,
                            )
                        exps[kc] = None

                    def sums_quad(qq):
                        if last and qq == NQUAD - 1:
                            # final q-tile: sum the last two pairs directly so
                            # the tail's critical chain skips the 1.25us
                            # GpSimd quad-add (exp31 -> pair15 -> matmul)
                            for j, pp in enumerate((NPAIR - 2, NPAIR - 1)):
                                nc.tensor.matmul(
                                    s_acc[:],
                                    ones_s[:],
                                    esums[pp][:],
                                    start=False,
                                    stop=(j == 1),
                                    skip_group_check=True,
                                )
                                esums[pp] = None
                            return
                        nc.tensor.matmul(
                            s_acc[:],
                            ones_s[:],
                            equads[qq][:],
                            start=(qq == 0),
                            stop=(qq == NQUAD - 1),
                            skip_group_check=True,
                        )
                        equads[qq] = None

                    epi_sched = {1: 0, 3: 1, 5: 2, 6: 3, 7: 4, 9: 5, 11: 6, 13: 7}
                    for kc in range(NKC + 6):
                        if prev is not None and kc in epi_sched:
                            emit_epilogue(prev, epi_sched[kc])
                        if kc < NKC:
                            scores_exp(kc)
                        if 2 <= kc < NKC + 2:
                            pv_only(kc - 2)
                        if kc >= 9 and (kc - 9) % 4 == 0 and (kc - 9) // 4 < NQUAD:
                            sums_quad((kc - 9) // 4)

                    prev = {"y_acc": y_acc, "s_acc": s_acc, "qsl": qsl,
                            "yT": [None] * NOC}

                # final q-tile's epilogue has no successor to hide in
                for step in range(8):
                    emit_epilogue(prev, step)
            all_ps.__exit__(None, None, None)
    return nc


def _get_nc():
    if "nc" not in _cache:
        _install_bir_patch()
        _cache["nc"] = _build_nc()
    return _cache["nc"]


def kernel(x, w_theta, b_theta, w_phi, b_phi, w_g, b_g, w_out, b_out,
           _trace=False):
    import ml_dtypes
    from concourse.bass_utils import run_bass_kernel_spmd

    x = np.asarray(x, dtype=np.float32)
    w_theta = np.asarray(w_theta, dtype=np.float32)
    b_theta = np.asarray(b_theta, dtype=np.float32)
    w_phi = np.asarray(w_phi, dtype=np.float32)
    b_phi = np.asarray(b_phi, dtype=np.float32)
    w_g = np.asarray(w_g, dtype=np.float32)
    b_g = np.asarray(b_g, dtype=np.float32)
    w_out = np.asarray(w_out, dtype=np.float32)
    b_out = np.asarray(b_out, dtype=np.float32)

    nc = _get_nc()

    xf = np.ascontiguousarray(x.reshape(B, C, N))
    wqp = np.ascontiguousarray(np.concatenate([w_theta, w_phi], axis=0).T)  # [C, 2CI]
    wg = np.ascontiguousarray(w_g.T.astype(ml_dtypes.bfloat16))  # [C, CI] bf16
    wo = np.ascontiguousarray(w_out.T)                     # [CI, C]
    bqp = np.ascontiguousarray(
        np.stack([b_theta[:P], b_theta[P:], b_phi[:P], b_phi[P:],
                  np.full(P, -SHIFT, np.float32),
                  np.full(P, -40.0 * np.log(2.0), np.float32)], axis=1))  # [P, 6]
    bo_eff = b_out + w_out @ b_g
    bo = np.ascontiguousarray(bo_eff.reshape(NCC, P).T)    # [P, NCC]
    ones = np.ones((P, P), dtype=ml_dtypes.bfloat16)

    shared = {"wqp": wqp, "wg": wg, "wo": wo, "bqp": bqp, "bo": bo, "ones": ones}
    in_maps = []
    for core in range(8):
        b, h = divmod(core, 2)
        # query half first; attention is permutation-invariant over keys
        xperm = np.concatenate(
            [xf[b][:, h * NQ:(h + 1) * NQ], xf[b][:, (1 - h) * NQ:(2 - h) * NQ]],
            axis=1)
        in_maps.append({"xf": np.ascontiguousarray(xperm), **shared})

    res = run_bass_kernel_spmd(nc, in_maps, core_ids=list(range(8)), trace=_trace)
    _cache["last_results"] = res

    out = np.empty((B, C, N), dtype=np.float32)
    for core in range(8):
        b, h = divmod(core, 2)
        out[b][:, h * NQ:(h + 1) * NQ] = res.results[core]["out"]
    return out.reshape(B, C, HH, WW)
